# revision 11
# baseline (speedup 1.0000x reference)
"""Trainium2 Bass kernel for nn_D_Attention_82377472738015.

Transformer decoder block: causal self-attention + cross-attention + FFN,
each with residual + layernorm.  B=8, S=1024, D=512, H=8, HD=64, DFF=2048.

Sharding: data-parallel over batch.  8 batch elements -> 8 NeuronCores,
weights replicated, no collectives.  Each core runs the full block on its
[1024, 512] slice.

Per-core layout: activations are kept TRANSPOSED ([feature, token]:
[128 part, D/128 chunks, S free]) so every projection is a natural
lhsT.T @ rhs with the feature dim contracting on partitions.  Attention
scores are computed transposed ([kv, q]) which makes the P@V contraction
natural; the softmax denominator comes from augmenting V with a ones
column, and the final divide uses a K=1 ones-outer-product matmul to
replicate 1/sum across partitions.
"""

import sys

sys.path.insert(0, "/opt/trn_rl_repo")

from contextlib import ExitStack

import numpy as np
import ml_dtypes

import concourse.bass as bass
import concourse.tile as tile
from concourse import bacc, mybir
from concourse.bass_utils import run_bass_kernel_spmd
from concourse.masks import make_identity

P = 128
S = 1024          # sequence length (per core)
D = 512           # model dim
H = 8             # heads
HD = 64           # head dim
DFF = 2048        # ffn hidden
DC = D // P       # 4 chunks of model dim
ST = S // P       # 8 tiles of sequence
SC = S // 512     # 2 free-dim chunks of 512
FC = DFF // P     # 16 chunks of ffn dim
EPS = 1e-5
NEG = -1e9
FP = mybir.dt.float32
BF = mybir.dt.bfloat16

NCORES = 8


def build(nc):
    AF = mybir.ActivationFunctionType

    # ---------------- DRAM parameters ----------------
    def din(name, shape, dt=FP):
        return nc.dram_tensor(name, shape, dt, kind="ExternalInput").ap()

    xd = din("x", [S, D], BF)
    fd = din("feature", [S, D], BF)
    wq_d, bq_d = din("wq", [D, D], BF), din("bq", [D])
    wk_d, bk_d = din("wk", [D, D], BF), din("bk", [D])
    wv_d, bv_d = din("wv", [D, D], BF), din("bv", [D])
    wo_d, bo_d = din("wo", [D, D], BF), din("bo", [D])
    ln1_g_d, ln1_b_d = din("ln1_g", [D]), din("ln1_b", [D])
    wqc_d, bqc_d = din("wqc", [D, D], BF), din("bqc", [D])
    wkc_d, bkc_d = din("wkc", [D, D], BF), din("bkc", [D])
    wvc_d, bvc_d = din("wvc", [D, D], BF), din("bvc", [D])
    woc_d, boc_d = din("woc", [D, D], BF), din("boc", [D])
    ln2_g_d, ln2_b_d = din("ln2_g", [D]), din("ln2_b", [D])
    w1_d, b1_d = din("w1", [D, DFF], BF), din("b1", [DFF])
    w2_d, b2_d = din("w2", [DFF, D], BF), din("b2", [D])
    lnf_g_d, lnf_b_d = din("lnf_g", [D]), din("lnf_b", [D])
    out_d = nc.dram_tensor("out", [S, D], FP, kind="ExternalOutput").ap()

    with tile.TileContext(nc) as tc, ExitStack() as top:
        const = top.enter_context(tc.tile_pool(name="const", bufs=1))
        # long-lived activation chain pool:
        #   tag "io" (bufs=2): xT, fT, outT   (outT rotates into xT's slot)
        #   tag "h"  (bufs=2): pre_s, h1, pre_c, h2, pre3 (rotating chain)
        chain = top.enter_context(tc.tile_pool(name="chain", bufs=1))

        # ---------------- constants ----------------
        ident = const.tile([P, P], FP, tag="ident", name="ident")
        make_identity(nc, ident)

        # causal mask bank Z: [NEG x384 | tri x128 | zeros x384].
        # For diagonal kv-tile u (0..3) within a 512-wide q chunk the mask is
        # Z[:, 384-128u : 896-128u]: adds NEG where kv_row > q_col.
        Z = const.tile([P, 896], FP, tag="Z", name="Z")
        nc.gpsimd.memset(Z[:, :], 0.0)
        nc.gpsimd.memset(Z[:, 0:384], NEG)
        # tri block: keep 0 where (col - row) >= 0, else NEG (masked: col < row)
        nc.gpsimd.affine_select(
            out=Z[:, 384:512],
            in_=Z[:, 384:512],
            compare_op=mybir.AluOpType.is_ge,
            fill=NEG,
            base=0,
            pattern=[[1, 128]],
            channel_multiplier=-1,
        )

        ones_col = const.tile([P, 1], FP, tag="ones_col", name="ones_col")
        nc.vector.memset(ones_col[:, :], 1.0)
        ones_row64 = const.tile([1, 64], FP, tag="ones_row64", name="ones_row64")
        nc.vector.memset(ones_row64[:, :], 1.0)
        ones_rowP = const.tile([1, P], FP, tag="ones_rowP", name="ones_rowP")
        nc.vector.memset(ones_rowP[:, :], 1.0)
        eps_col = const.tile([P, 1], FP, tag="eps_col", name="eps_col")
        nc.vector.memset(eps_col[:, :], EPS)
        ident_bf = const.tile([P, P], BF, tag="ident_bf", name="ident_bf")
        nc.vector.tensor_copy(ident_bf[:, :], ident[:, :])
        ones_rowP_bf = const.tile([1, P], BF, tag="ones_rowP_bf", name="ones_rowP_bf")
        nc.vector.memset(ones_rowP_bf[:, :], 1.0)

        def big(pool, tag, name, bufs=None, dt=BF):
            return pool.tile([P, DC, S], dt, tag=tag, name=name, bufs=bufs)

        # ---------------- small helpers ----------------
        def load_w(dram, K, N, pool, tag):
            """[K, N] dram -> [P, K//P, N] sbuf (partition = K % P)."""
            t = pool.tile([P, K // P, N], BF, tag=tag, name=tag)
            nc.sync.dma_start(t[:], dram.rearrange("(c p) n -> p c n", p=P))
            return t

        def load_bias_part(dram, K, pool, tag):
            """[K] dram -> [P, K//P] sbuf (per-partition vector chunks)."""
            t = pool.tile([P, K // P], FP, tag=tag, name=tag)
            nc.sync.dma_start(t[:], dram.rearrange("(c p) -> p c", p=P))
            return t

        def load_bias_row(dram, N, pool, tag):
            """[N] dram -> [1, N] bf16 sbuf row (for the K=1 bias-init matmul)."""
            t32 = pool.tile([1, N], FP, tag=tag + "32", name=tag + "32")
            nc.sync.dma_start(t32[:], dram.rearrange("(a n) -> a n", a=1))
            t = pool.tile([1, N], BF, tag=tag, name=tag)
            nc.vector.tensor_copy(t[:, :], t32[:, :])
            return t

        def load_T(dram_ap, name):
            """[S, D] dram -> transposed [P, DC, S] sbuf via PE transposes."""
            dst = big(chain, "io", name, bufs=2)
            with tc.tile_pool(name=f"ld_{name}", bufs=3) as ldp, tc.tile_pool(
                name=f"ldps_{name}", bufs=4, space="PSUM"
            ) as pps:
                for ti in range(ST):
                    nat = ldp.tile([P, D], BF, tag="nat", name=f"nat_{name}_{ti}")
                    nc.sync.dma_start(nat[:], dram_ap[ti * P : (ti + 1) * P, :])
                    for c in range(DC):
                        ps = pps.tile([P, P], BF, tag="tp", name=f"tp_{name}_{ti}_{c}")
                        nc.tensor.transpose(ps[:], nat[:, c * P : (c + 1) * P], ident_bf)
                        nc.scalar.copy(dst[:, c, ti * P : (ti + 1) * P], ps[:])
            return dst

        def linear_T(in_T, w_sb, bias_part, outT, name, res_T=None):
            """outT[P, DC, S] = w^T @ in_T + bias (+ res_T).  All in T layout."""
            with tc.tile_pool(name=f"ps_{name}", bufs=3, space="PSUM") as pp:
                for m in range(DC):
                    for sc in range(SC):
                        ps = pp.tile([P, 512], FP, tag="proj", name=f"ps_{name}_{m}_{sc}")
                        for c in range(DC):
                            nc.tensor.matmul(
                                ps[:],
                                lhsT=w_sb[:, c, m * P : (m + 1) * P],
                                rhs=in_T[:, c, sc * 512 : (sc + 1) * 512],
                                start=(c == 0),
                                stop=(c == DC - 1),
                            )
                        o = outT[:, m, sc * 512 : (sc + 1) * 512]
                        nc.scalar.add(o, ps[:], bias_part[:, m : m + 1])
                        if res_T is not None:
                            nc.vector.tensor_add(
                                o, o, res_T[:, m, sc * 512 : (sc + 1) * 512]
                            )
            return outT

        def v_natural(in_T, wv_sb, bv_row, pool, name):
            """V in natural layout + ones column: [P, ST, H, HD+1]."""
            V = pool.tile([P, ST, H, HD + 1], BF, tag="v", name=name, bufs=1)
            nc.vector.memset(V[:, :, :, HD], 1.0)
            with tc.tile_pool(name=f"ps_{name}", bufs=3, space="PSUM") as pp:
                for kt in range(ST):
                    ps = pp.tile([P, 512], FP, tag="v", name=f"ps_{name}_{kt}")
                    # bias init: psum[p, n] = bv[n] via K=1 ones matmul
                    nc.tensor.matmul(
                        ps[:], lhsT=ones_rowP_bf[:, :], rhs=bv_row[:, :],
                        start=True, stop=False,
                    )
                    for c in range(DC):
                        nc.tensor.matmul(
                            ps[:],
                            lhsT=in_T[:, c, kt * P : (kt + 1) * P],
                            rhs=wv_sb[:, c, :],
                            start=False,
                            stop=(c == DC - 1),
                        )
                    nc.vector.tensor_copy(
                        out=V[:, kt, :, 0:HD],
                        in_=ps[:].rearrange("p (h d) -> p h d", h=H),
                    )
            return V

        def layernorm_T(inT, g_sb, b_sb, outT, name):
            """LN over the feature dim (partitions x DC chunks) in T layout."""
            with tc.tile_pool(name=f"lnps_{name}", bufs=1, space="PSUM") as pp, \
                 tc.tile_pool(name=f"lnsb_{name}", bufs=2) as sb:
                for sc in range(SC):
                    sl = slice(sc * 512, (sc + 1) * 512)
                    psA = pp.tile([P, 512], FP, tag="A", name=f"lnA_{name}_{sc}")
                    psB = pp.tile([P, 512], FP, tag="B", name=f"lnB_{name}_{sc}")
                    for c in range(DC):
                        # sums (plain f32 matmul for stats robustness)
                        nc.tensor.matmul(
                            psA[0:1, :], lhsT=ones_col[:, :], rhs=inT[:, c, sl],
                            start=(c == 0), stop=(c == DC - 1),
                        )
                    for c in range(DC):
                        sq = sb.tile([P, 512], FP, tag="sq", name=f"lnsq_{name}_{sc}_{c}")
                        nc.vector.tensor_tensor(
                            sq[:], inT[:, c, sl], inT[:, c, sl], mybir.AluOpType.mult
                        )
                        nc.tensor.matmul(
                            psB[0:1, :], lhsT=ones_col[:, :], rhs=sq[:],
                            start=(c == 0), stop=(c == DC - 1),
                        )
                    mu = sb.tile([1, 512], FP, tag="mu", name=f"lnmu_{name}_{sc}")
                    nc.vector.tensor_scalar_mul(mu[:, :], psA[0:1, :], 1.0 / D)
                    ex2 = sb.tile([1, 512], FP, tag="ex2", name=f"lnex2_{name}_{sc}")
                    nc.vector.tensor_scalar_mul(ex2[:, :], psB[0:1, :], 1.0 / D)
                    var = sb.tile([1, 512], FP, tag="var", name=f"lnvar_{name}_{sc}")
                    nc.vector.tensor_tensor(var[:, :], mu[:, :], mu[:, :], mybir.AluOpType.mult)
                    nc.vector.tensor_tensor(var[:, :], ex2[:, :], var[:, :], mybir.AluOpType.subtract)
                    std = sb.tile([1, 512], FP, tag="std", name=f"lnstd_{name}_{sc}")
                    nc.scalar.activation(std[:, :], var[:, :], AF.Sqrt, bias=eps_col[0:1, :])
                    rs = sb.tile([1, 512], FP, tag="rs", name=f"lnrs_{name}_{sc}")
                    nc.vector.reciprocal(rs[:, :], std[:, :])
                    murs = sb.tile([1, 512], FP, tag="murs", name=f"lnmurs_{name}_{sc}")
                    nc.vector.tensor_tensor(murs[:, :], mu[:, :], rs[:, :], mybir.AluOpType.mult)
                    # replicate rs, mu*rs across partitions via K=1 matmuls
                    psR = pp.tile([P, 512], FP, tag="R", name=f"lnR_{name}_{sc}")
                    nc.tensor.matmul(psR[:, :], lhsT=ones_rowP[:, :], rhs=rs[:, :],
                                     start=True, stop=True)
                    rs_rep = sb.tile([P, 512], FP, tag="rs_rep", name=f"lnrsrep_{name}_{sc}")
                    nc.scalar.copy(rs_rep[:, :], psR[:, :])
                    psM = pp.tile([P, 512], FP, tag="M", name=f"lnM_{name}_{sc}")
                    nc.tensor.matmul(psM[:, :], lhsT=ones_rowP[:, :], rhs=murs[:, :],
                                     start=True, stop=True)
                    murs_rep = sb.tile([P, 512], FP, tag="murs_rep", name=f"lnmursrep_{name}_{sc}")
                    nc.scalar.copy(murs_rep[:, :], psM[:, :])
                    for c in range(DC):
                        t1 = sb.tile([P, 512], FP, tag="t1", name=f"lnt1_{name}_{sc}_{c}")
                        nc.vector.tensor_tensor(
                            t1[:], inT[:, c, sl], rs_rep[:, :], mybir.AluOpType.mult
                        )
                        nc.vector.tensor_tensor(
                            t1[:], t1[:], murs_rep[:, :], mybir.AluOpType.subtract
                        )
                        # out = t1 * g + b  (per-partition scale/bias)
                        nc.scalar.activation(
                            outT[:, c, sl], t1[:], AF.Identity,
                            bias=b_sb[:, c : c + 1], scale=g_sb[:, c : c + 1],
                        )
            return outT

        def attention(xqT, kvT, acts, w, g_sb, b_sb, causal, blk):
            """One MHA block + residual + LN.  Returns LN output tile."""
            (wq_sb, bq_sb, wk_sb, bk_sb, wv_sb, bv_row, wo_sb, bo_sb) = w
            QT = big(acts, "qk", f"QT{blk}", bufs=2)
            linear_T(xqT, wq_sb, bq_sb, QT, f"QT{blk}")
            KT = big(acts, "qk", f"KT{blk}", bufs=2)
            linear_T(kvT, wk_sb, bk_sb, KT, f"KT{blk}")
            V = v_natural(kvT, wv_sb, bv_row, acts, f"V{blk}")
            OT = big(acts, "o", f"OT{blk}", bufs=1)
            with tc.tile_pool(name=f"att_sb{blk}", bufs=3) as sb, \
                 tc.tile_pool(name=f"att_ps{blk}", bufs=3, space="PSUM") as psc, \
                 tc.tile_pool(name=f"att_pv{blk}", bufs=2, space="PSUM") as ppv, \
                 tc.tile_pool(name=f"att_pr{blk}", bufs=2, space="PSUM") as prp:
                for h in range(H):
                    bp = (h % 2) * 64
                    mt = h // 2
                    for sc in range(SC):
                        qsl = slice(sc * 512, (sc + 1) * 512)
                        n_kv = (4 * sc + 4) if causal else ST
                        pv = ppv.tile([P, 512], FP, tag="pv", name=f"pv{blk}_{h}_{sc}")
                        for j in range(n_kv):
                            sps = psc.tile([P, 512], FP, tag="s", name=f"s{blk}_{h}_{sc}_{j}")
                            nc.tensor.matmul(
                                sps[:],
                                lhsT=KT[bp : bp + 64, mt, j * P : (j + 1) * P],
                                rhs=QT[bp : bp + 64, mt, qsl],
                                start=True, stop=True,
                            )
                            if causal and j >= 4 * sc:
                                u = j - 4 * sc
                                nc.vector.tensor_add(
                                    sps[:], sps[:], Z[:, 384 - 128 * u : 896 - 128 * u]
                                )
                            pT = sb.tile([P, 512], BF, tag="pT", name=f"pT{blk}_{h}_{sc}_{j}")
                            nc.scalar.activation(pT[:], sps[:], AF.Exp)
                            nc.tensor.matmul(
                                pv[0 : HD + 1, :],
                                lhsT=V[:, j, h, :],
                                rhs=pT[:],
                                start=(j == 0),
                                stop=(j == n_kv - 1),
                            )
                        recip = sb.tile([1, 512], FP, tag="recip", name=f"rc{blk}_{h}_{sc}")
                        nc.vector.reciprocal(recip[:, :], pv[HD : HD + 1, :])
                        psR = prp.tile([64, 512], FP, tag="rep", name=f"rep{blk}_{h}_{sc}")
                        nc.tensor.matmul(psR[:, :], lhsT=ones_row64[:, :], rhs=recip[:, :],
                                         start=True, stop=True)
                        reprow = sb.tile([64, 512], FP, tag="reprow", name=f"rr{blk}_{h}_{sc}")
                        nc.scalar.copy(reprow[:, :], psR[:, :])
                        nc.vector.tensor_tensor(
                            OT[bp : bp + 64, mt, qsl], pv[0:HD, :], reprow[:, :],
                            mybir.AluOpType.mult,
                        )
            pre = big(chain, "h", f"pre{blk}", bufs=2, dt=FP)
            linear_T(OT, wo_sb, bo_sb, pre, f"pre{blk}", res_T=xqT)
            hout = big(chain, "h", f"h{blk}", bufs=2)
            layernorm_T(pre, g_sb, b_sb, hout, f"h{blk}")
            return hout

        # ---------------- phase A: inputs ----------------
        xT = load_T(xd, "xT")
        fT = load_T(fd, "fT")

        # ---------------- attention phases ----------------
        with ExitStack() as attn_scope:
            acts = attn_scope.enter_context(tc.tile_pool(name="attn_acts", bufs=1))

            with tc.tile_pool(name="attw1", bufs=1) as wp:
                w = (
                    load_w(wq_d, D, D, wp, "wq"),
                    load_bias_part(bq_d, D, wp, "bq"),
                    load_w(wk_d, D, D, wp, "wk"),
                    load_bias_part(bk_d, D, wp, "bk"),
                    load_w(wv_d, D, D, wp, "wv"),
                    load_bias_row(bv_d, D, wp, "bv"),
                    load_w(wo_d, D, D, wp, "wo"),
                    load_bias_part(bo_d, D, wp, "bo"),
                )
                g1 = load_bias_part(ln1_g_d, D, wp, "g1")
                b1n = load_bias_part(ln1_b_d, D, wp, "b1n")
                h1T = attention(xT, xT, acts, w, g1, b1n, causal=True, blk="s")

            with tc.tile_pool(name="attw2", bufs=1) as wp:
                w = (
                    load_w(wqc_d, D, D, wp, "wqc"),
                    load_bias_part(bqc_d, D, wp, "bqc"),
                    load_w(wkc_d, D, D, wp, "wkc"),
                    load_bias_part(bkc_d, D, wp, "bkc"),
                    load_w(wvc_d, D, D, wp, "wvc"),
                    load_bias_row(bvc_d, D, wp, "bvc"),
                    load_w(woc_d, D, D, wp, "woc"),
                    load_bias_part(boc_d, D, wp, "boc"),
                )
                g2 = load_bias_part(ln2_g_d, D, wp, "g2")
                b2n = load_bias_part(ln2_b_d, D, wp, "b2n")
                h2T = attention(h1T, fT, acts, w, g2, b2n, causal=False, blk="c")

        # ---------------- FFN ----------------
        with tc.tile_pool(name="ffnw", bufs=1) as wp:
            w1_sb = load_w(w1_d, D, DFF, wp, "w1")       # [P, 4, 2048]
            w2_sb = load_w(w2_d, DFF, D, wp, "w2")       # [P, 16, 512]
            b1_sb = load_bias_part(b1_d, DFF, wp, "b1")  # [P, 16]
            b2_sb = load_bias_part(b2_d, D, wp, "b2")    # [P, 4]
            gf = load_bias_part(lnf_g_d, D, wp, "gf")
            bf = load_bias_part(lnf_b_d, D, wp, "bf")
            pre3 = big(chain, "h", "pre3", bufs=2, dt=FP)
            with tc.tile_pool(name="ffn_sb", bufs=3) as sb, \
                 tc.tile_pool(name="ffn_ps2", bufs=1, space="PSUM") as pf2, \
                 tc.tile_pool(name="ffn_ps1", bufs=3, space="PSUM") as pf1:
                for sc in range(SC):
                    sl = slice(sc * 512, (sc + 1) * 512)
                    ff2ps = [
                        pf2.tile([P, 512], FP, tag=f"m{m}", name=f"ff2_{sc}_{m}")
                        for m in range(DC)
                    ]
                    for f in range(FC):
                        fps = pf1.tile([P, 512], FP, tag="f1", name=f"ff1_{sc}_{f}")
                        for c in range(DC):
                            nc.tensor.matmul(
                                fps[:],
                                lhsT=w1_sb[:, c, f * P : (f + 1) * P],
                                rhs=h2T[:, c, sl],
                                start=(c == 0),
                                stop=(c == DC - 1),
                            )
                        ff1 = sb.tile([P, 512], BF, tag="ff1", name=f"ff1sb_{sc}_{f}")
                        nc.scalar.activation(
                            ff1[:], fps[:], AF.Relu, bias=b1_sb[:, f : f + 1]
                        )
                        for m in range(DC):
                            nc.tensor.matmul(
                                ff2ps[m][:],
                                lhsT=w2_sb[:, f, m * P : (m + 1) * P],
                                rhs=ff1[:],
                                start=(f == 0),
                                stop=(f == FC - 1),
                            )
                    for m in range(DC):
                        o = pre3[:, m, sl]
                        nc.scalar.add(o, ff2ps[m][:], b2_sb[:, m : m + 1])
                        nc.vector.tensor_add(o, o, h2T[:, m, sl])
            outT = big(chain, "io", "outT", bufs=2, dt=FP)
            layernorm_T(pre3, gf, bf, outT, "outT")

        # ---------------- output transpose + DMA ----------------
        with tc.tile_pool(name="out_sb", bufs=3) as sb, \
             tc.tile_pool(name="out_ps", bufs=4, space="PSUM") as pp:
            for ti in range(ST):
                onat = sb.tile([P, D], FP, tag="onat", name=f"onat_{ti}")
                for m in range(DC):
                    ps = pp.tile([P, P], FP, tag="tp", name=f"otp_{ti}_{m}")
                    nc.tensor.transpose(ps[:], outT[:, m, ti * P : (ti + 1) * P], ident)
                    nc.scalar.copy(onat[:, m * P : (m + 1) * P], ps[:])
                nc.sync.dma_start(out_d[ti * P : (ti + 1) * P, :], onat[:])

    return nc


_CACHE = {}


def _get_graph():
    if "nc" not in _CACHE:
        nc = bacc.Bacc(
            "TRN2", target_bir_lowering=False, debug=False, num_devices=NCORES
        )
        build(nc)
        nc.compile()
        _CACHE["nc"] = nc
    return _CACHE["nc"]


def kernel(**inputs):
    nc = _get_graph()
    scale = 1.0 / np.sqrt(np.float32(D))

    BF_NP = ml_dtypes.bfloat16
    BF_KEYS = {"wq", "wk", "wv", "wo", "wqc", "wkc", "wvc", "woc", "w1", "w2"}
    weights = {}
    for k, v in inputs.items():
        if k in ("x", "feature"):
            continue
        weights[k] = np.ascontiguousarray(np.asarray(v, dtype=np.float32))
    # fold the 1/sqrt(D) score scaling into the query projections
    weights["wq"] = weights["wq"] * scale
    weights["bq"] = weights["bq"] * scale
    weights["wqc"] = weights["wqc"] * scale
    weights["bqc"] = weights["bqc"] * scale
    for k in BF_KEYS:
        weights[k] = weights[k].astype(BF_NP)

    x = np.ascontiguousarray(np.asarray(inputs["x"], dtype=np.float32).astype(BF_NP))
    feature = np.ascontiguousarray(
        np.asarray(inputs["feature"], dtype=np.float32).astype(BF_NP)
    )

    in_maps = []
    for i in range(NCORES):
        m = dict(weights)
        m["x"] = x[i]
        m["feature"] = feature[i]
        in_maps.append(m)

    import os

    trace = bool(int(os.environ.get("KERNEL_TRACE", "0")))
    kw = {}
    if trace:
        kw["trace"] = True
        kw["tmpdir"] = os.environ.get("KERNEL_TRACE_DIR") or None
    res = run_bass_kernel_spmd(nc, in_maps, core_ids=list(range(NCORES)), **kw)
    if trace:
        print(f"HW exec time: {res.exec_time_ns} ns")
        _CACHE["exec_time_ns"] = res.exec_time_ns
        _CACHE["profile_json"] = res.profile_json
    out = np.stack([res.results[i]["out"] for i in range(NCORES)], axis=0)
    return out, inputs["feature"]


if __name__ == "__main__":
    # smoke build
    _get_graph()
    print("graph built OK")
